# revision 5
# baseline (speedup 1.0000x reference)
"""Trainium2 Bass kernel for the EnforcedNeuralODE recurrence.

Reference computation (per timestep):
    x_t = fc_w @ concat(x_{t-1}, f_{t-1}) + fc_b
      i.e. x_t = Wx x_{t-1} + Wf f_{t-1} + b
over T-1 = 4095 steps, batch 256, state 64, force 64.
Output: [T, B, 64] = concat([x_0], [x_1..x_{T-1}]).

Strategy: data-parallel batch shard (32 samples/core across 8 cores); on
each core a blocked parallel scan over K=32-step blocks:
  P1: within-block prefixes (odd steps only, unroll-2 chain), batched
      across the chunk's blocks in the matmul free dim:
        h_{2p+1} = Wx^2 h_{2p-1} + (Wx Wf) f_{2p} + Wf f_{2p+1} + (Wx b + b)
  P2: block-boundary scan (128 sequential tiny steps):
        s_{b+1} = Wx^K s_b + h_{K-1}
  P3: combine (embarrassingly parallel):
        even j: X_j = Wx^{j+1} s + Wx h_{j-1} + Wf f_j + b
        odd  j: X_j = Wx^{j+1} s + h_j
Matrix powers/products precomputed on host (f64, cast f32).

Hardware constraints honored:
  - Every matmul uses contraction rows 0..64 (K=65; row 64 is a host-
    provided zeros row for F/H, a ones row for S so the bias rides the
    matmul as lhsT row 64).  Mixing operand partition-halves between
    matmuls that share PSUM partitions crashes the device
    (NRT_EXEC_UNIT_UNRECOVERABLE), so everything stays on one half;
    uniform (rows, 64-col) tiling mode also avoids PE drain thrash.
  - PSUM column tiles (0,0)/(0,64) pack even/odd steps into one
    [128, N] psum tile so the PSUM->SBUF evacuation runs 128 wide.
"""

import numpy as np
from contextlib import ExitStack

NCORES = 8
BATCH, STATE, FDIM, TIMESPAN = 256, 64, 64, 4096

# per-core tiling
BC = BATCH // NCORES        # 32 batch per core
K = 32                      # steps per block
PAIRS = K // 2              # 16
NB = TIMESPAN // K          # 128 blocks (steps padded 4095 -> 4096)
NBC = 8                     # blocks per chunk
CHUNKS = NB // NBC          # 16
N = NBC * BC                # 256 free-dim per step column
F_COLS = PAIRS * 2 * N      # 8192 forcing cols per chunk (both parities)
H_COLS = PAIRS * N          # 4096 prefix cols per chunk
O_COLS = PAIRS * N          # 4096 output cols per chunk (pair-packed)

_NC_CACHE: dict = {}


def _set_dims(ncores=8, bc=32, k=32, nbc=8, chunks=16):
    """Override problem dims (testing only). Recomputes derived globals."""
    global NCORES, BATCH, BC, K, PAIRS, NB, NBC, CHUNKS, N
    global F_COLS, H_COLS, O_COLS, TIMESPAN
    NCORES, BC, K, NBC, CHUNKS = ncores, bc, k, nbc, chunks
    BATCH = NCORES * BC
    PAIRS = K // 2
    NB = CHUNKS * NBC
    TIMESPAN = NB * K
    N = NBC * BC
    F_COLS = PAIRS * 2 * N
    H_COLS = PAIRS * N
    O_COLS = PAIRS * N


def _build_nc(chunks, nbc, bc, k):
    """Build + compile the per-core Bass module (SPMD: same NEFF all cores)."""
    import concourse.bass as bass  # noqa: F401
    import concourse.tile as tile
    from concourse import bacc, mybir

    pairs = k // 2
    n = nbc * bc
    f_cols = pairs * 2 * n
    h_cols = pairs * n
    o_cols = pairs * n
    nb = chunks * nbc
    f32 = mybir.dt.float32
    AF = mybir.ActivationFunctionType

    nc = bacc.Bacc("TRN2", target_bir_lowering=False, debug=False)

    f_dram = nc.dram_tensor("f", [65, chunks * f_cols], f32, kind="ExternalInput")
    wpow_dram = nc.dram_tensor("wpow", [65, k * 64], f32, kind="ExternalInput")
    wsml_dram = nc.dram_tensor("wsml", [65, 336], f32, kind="ExternalInput")
    s0_dram = nc.dram_tensor("s0", [64, bc], f32, kind="ExternalInput")
    zrow_dram = nc.dram_tensor("zrow", [1, h_cols], f32, kind="ExternalInput")
    out_dram = nc.dram_tensor("out", [128, chunks * o_cols], f32, kind="ExternalOutput")

    with tile.TileContext(nc) as tc, ExitStack() as ctx:
        singles = ctx.enter_context(tc.tile_pool(name="singles", bufs=1))
        fpool = ctx.enter_context(tc.tile_pool(name="fpool", bufs=4))
        hpool = ctx.enter_context(tc.tile_pool(name="hpool", bufs=2))
        opool = ctx.enter_context(tc.tile_pool(name="opool", bufs=3))
        p1ps = ctx.enter_context(tc.tile_pool(name="p1ps", bufs=2, space="PSUM"))
        p3ps = ctx.enter_context(tc.tile_pool(name="p3ps", bufs=3, space="PSUM"))
        p2ps = ctx.enter_context(tc.tile_pool(name="p2ps", bufs=2, space="PSUM"))

        wpow = singles.tile([65, k * 64], f32)
        nc.sync.dma_start(out=wpow[:], in_=wpow_dram[:])
        wsml = singles.tile([65, 336], f32)
        nc.sync.dma_start(out=wsml[:], in_=wsml_dram[:])
        # block start states: [65, (nb+1)*bc]; row 64 = ones (bias row)
        s65 = singles.tile([65, (nb + 1) * bc], f32)
        nc.vector.memset(s65[64:65, :], 1.0)
        nc.sync.dma_start(out=s65[0:64, 0:bc], in_=s0_dram[:])

        # weight slices inside wsml (columns), all [65, 64], row 64 zero:
        a_wx2 = wsml[:, 0:64]     # (Wx^2)^T
        a_wxwf = wsml[:, 64:128]  # (Wx Wf)^T
        a_wx = wsml[:, 128:192]   # Wx^T
        a_wf = wsml[:, 192:256]   # Wf^T
        a_eye = wsml[:, 256:320]  # I
        b2_ap = wsml[0:64, 320:321]  # Wx b + b (bias for P1 copies)

        fh_cols = f_cols // 2      # forcing cols per half-chunk tile
        fh_pairs = pairs // 2      # pairs per F tile
        os_pairs = pairs // 2      # pairs per out-stage tile
        os_cols = os_pairs * n

        for c in range(chunks):
            ftiles = []
            for fh in range(2):
                ft = fpool.tile([65, fh_cols], f32, tag="F")
                nc.sync.dma_start(
                    out=ft[:],
                    in_=f_dram[:, c * f_cols + fh * fh_cols : c * f_cols + (fh + 1) * fh_cols],
                )
                ftiles.append(ft)

            htile = hpool.tile([65, h_cols], f32, tag="H")
            nc.sync.dma_start(out=htile[64:65, :], in_=zrow_dram[:])

            def fslice(p, parity):
                ft = ftiles[p // fh_pairs]
                base = (p % fh_pairs) * 2 * n + parity * n
                return ft[:, base : base + n]

            # ---- P1: within-block odd prefixes (sequential chain) ----
            for p in range(pairs):
                ps = p1ps.tile([64, n], f32)
                nc.tensor.matmul(ps[:], a_wxwf, fslice(p, 0), start=True, stop=False)
                if p > 0:
                    nc.tensor.matmul(
                        ps[:], a_wx2, htile[:, (p - 1) * n : p * n],
                        start=False, stop=False,
                    )
                nc.tensor.matmul(ps[:], a_wf, fslice(p, 1), start=False, stop=True)
                # h = psum + b2   (ScalarE, PSUM->SBUF with per-partition bias)
                nc.scalar.activation(
                    htile[0:64, p * n : (p + 1) * n], ps[:], AF.Identity, bias=b2_ap
                )

            # ---- P2: block-boundary scan for this chunk's blocks ----
            for blk in range(nbc):
                bg = c * nbc + blk
                ps2 = p2ps.tile([64, bc], f32)
                nc.tensor.matmul(
                    ps2[:],
                    wpow[:, (k - 1) * 64 : k * 64],
                    s65[:, bg * bc : (bg + 1) * bc],
                    start=True, stop=False,
                )
                nc.tensor.matmul(
                    ps2[:],
                    a_eye,
                    htile[:, (pairs - 1) * n + blk * bc : (pairs - 1) * n + (blk + 1) * bc],
                    start=False, stop=True,
                )
                nc.scalar.activation(
                    s65[0:64, (bg + 1) * bc : (bg + 2) * bc], ps2[:], AF.Copy
                )

            # ---- P3: combine + write out ----
            scol = s65[:, c * n : (c + 1) * n]
            for ohalf in range(2):
                ostage = opool.tile([128, os_cols], f32, tag="OS")
                for pp in range(os_pairs):
                    p = ohalf * os_pairs + pp
                    j0, j1 = 2 * p, 2 * p + 1
                    px = p3ps.tile([128, n], f32)
                    # even step -> psum partitions 0:64 (col tile 0)
                    nc.tensor.matmul(
                        px[0:64, :], wpow[:, j0 * 64 : (j0 + 1) * 64], scol,
                        start=True, stop=False,
                    )
                    if p > 0:
                        nc.tensor.matmul(
                            px[0:64, :], a_wx, htile[:, (p - 1) * n : p * n],
                            start=False, stop=False,
                        )
                    nc.tensor.matmul(
                        px[0:64, :], a_wf, fslice(p, 0), start=False, stop=True
                    )
                    # odd step -> psum partitions 64:128 (col tile 64)
                    nc.tensor.matmul(
                        px[64:128, :], wpow[:, j1 * 64 : (j1 + 1) * 64], scol,
                        start=True, stop=False,
                    )
                    nc.tensor.matmul(
                        px[64:128, :], a_eye, htile[:, p * n : (p + 1) * n],
                        start=False, stop=True,
                    )
                    nc.vector.tensor_copy(ostage[:, pp * n : (pp + 1) * n], px[:])
                nc.sync.dma_start(
                    out=out_dram[:, c * o_cols + ohalf * os_cols : c * o_cols + (ohalf + 1) * os_cols],
                    in_=ostage[:],
                )

    nc.compile()
    return nc


def _get_nc():
    key = (CHUNKS, NBC, BC, K)
    if key not in _NC_CACHE:
        _NC_CACHE[key] = _build_nc(CHUNKS, NBC, BC, K)
    return _NC_CACHE[key]


def _host_prep(inputs, forcing, fc_w, fc_b):
    """Build per-core input maps (numpy only, untimed)."""
    S = STATE
    fc_w = np.asarray(fc_w, np.float32)
    fc_b = np.asarray(fc_b, np.float32)
    Wx = fc_w[:, :S].astype(np.float64)
    Wf = fc_w[:, S:].astype(np.float64)
    b = fc_b.astype(np.float64)

    wsml = np.zeros((65, 336), np.float32)
    wsml[0:64, 0:64] = (Wx @ Wx).T.astype(np.float32)
    wsml[0:64, 64:128] = (Wx @ Wf).T.astype(np.float32)
    wsml[0:64, 128:192] = Wx.T.astype(np.float32)
    wsml[0:64, 192:256] = Wf.T.astype(np.float32)
    wsml[0:64, 256:320] = np.eye(64, dtype=np.float32)
    wsml[0:64, 320] = (Wx @ b + b).astype(np.float32)

    # wpow: col block j holds (Wx^{j+1})^T; row 64 = b for even j else 0
    wpow = np.zeros((65, K * 64), np.float32)
    P = np.eye(S, dtype=np.float64)
    for j in range(K):
        P = Wx @ P
        wpow[0:64, j * 64 : (j + 1) * 64] = P.T.astype(np.float32)
        if j % 2 == 0:
            wpow[64, j * 64 : (j + 1) * 64] = b.astype(np.float32)

    # forcing: [T-1, B, F] -> pad -> [65, c, p, parity, blk, bcore] cols
    steps = TIMESPAN
    fpad = np.zeros((steps, BATCH, FDIM), np.float32)
    fpad[: TIMESPAN - 1] = np.asarray(forcing, np.float32)
    # t = (c*NBC + blk)*K + 2p + parity
    arr = fpad.reshape(CHUNKS, NBC, PAIRS, 2, BATCH, FDIM)
    arr = arr.transpose(5, 0, 2, 3, 1, 4)  # [feat, c, p, parity, blk, bfull]

    inputs = np.asarray(inputs, np.float32)
    zrow = np.zeros((1, H_COLS), np.float32)
    in_maps = []
    for core in range(NCORES):
        bs = slice(core * BC, (core + 1) * BC)
        fcore = np.zeros((65, CHUNKS * F_COLS), np.float32)
        fcore[0:64] = np.ascontiguousarray(arr[..., bs]).reshape(64, CHUNKS * F_COLS)
        s0 = np.ascontiguousarray(inputs[bs].T)  # [64, BC]
        in_maps.append(
            {"f": fcore, "wpow": wpow, "wsml": wsml, "s0": s0, "zrow": zrow}
        )
    return in_maps


def _host_decode(results, inputs):
    """Per-core out [128, CHUNKS*O_COLS] -> full [T, B, S]."""
    inputs = np.asarray(inputs, np.float32)
    out = np.empty((TIMESPAN, BATCH, STATE), np.float32)
    out[0] = inputs
    for core in range(NCORES):
        o = results[core]["out"].reshape(2, 64, CHUNKS, PAIRS, NBC, BC)
        # [parity, s, c, p, blk, b] -> [c, blk, p, parity, b, s]
        o = o.transpose(2, 4, 3, 0, 5, 1).reshape(TIMESPAN, BC, STATE)
        out[1:, core * BC : (core + 1) * BC] = o[: TIMESPAN - 1]
    return out


def kernel(inputs, forcing, fc_w, fc_b, timespan):
    from concourse.bass_utils import run_bass_kernel_spmd

    timespan = int(timespan)
    assert timespan == TIMESPAN, f"hardcoded for timespan={TIMESPAN}, got {timespan}"
    nc = _get_nc()
    in_maps = _host_prep(inputs, forcing, fc_w, fc_b)
    res = run_bass_kernel_spmd(nc, in_maps, core_ids=list(range(NCORES)))
    return _host_decode(res.results, inputs)


if __name__ == "__main__":
    nc = _get_nc()
    print("built ok")
